# revision 6
# baseline (speedup 1.0000x reference)
"""GCN 2-hop (GCNConv + relu + residual + LayerNorm, twice) on 8 TRN2
NeuronCores via Bass/Tile — self-contained.

Distribution: dst-node partitioning across the 8 cores (graph parallel).
Each core owns a degree-balanced set of 2500 destination nodes (20 blocks
of <=128). Per core:
  hop1: agg1 = segment_sum(norm_e * x[src_e]) using dma_gather of x rows +
        PSUM-accumulated matmuls against host-built sparse (norm-valued)
        scatter tiles; delta = relu(agg1 @ W1 + b1);
        h = LN(x_own @ Wres + bres + delta) * g1 + be1.
        (linearity: segment_sum(norm * (x@W)[src]) == segment_sum(norm*x[src]) @ W)
  exchange: chunked AllGather of h (bf16) -> h_full, chunks interleaved into
        hop1 so most of the exchange overlaps compute.
  hop2: agg2 = segment_sum(norm_e * h_full[src_e]) the same way (256 wide);
        out = LN(h_own + agg2 @ W2 + b2) * g2 + be2.

Per dst block: edges whose src is assigned to the same (core, block) bin
(all self loops included) skip the gather entirely (direct matmul against
x rows / h_own with a diag tile); the rest dedup per (src, block) - one
gathered row per distinct src, scatter-tile rows may carry several cols.

Falls back to a pure-numpy implementation if the device path fails.
"""
import sys

for _p in ("/opt/trn_rl_repo",):
    if _p not in sys.path:
        sys.path.insert(0, _p)

import numpy as np
import ml_dtypes

P = 128
LN_EPS = 1e-5
N, E, DIN, DH, NCORE = 20000, 320000, 128, 256, 8
AG_CHUNKS, BPC = 4, 5
CHUNK_BLOCKS = [(0, 7), (7, 6), (13, 6), (19, 1)]  # (first_block, n_blocks)


def _make_cfg():
    NPC = N // NCORE
    nblk = (NPC + P - 1) // P
    blk_rows = [min(P, NPC - b * P) for b in range(nblk)]
    return dict(N=N, E=E, DIN=DIN, DH=DH, NCORE=NCORE, NPC=NPC,
                nblk=nblk, blk_rows=blk_rows, ag_chunks=AG_CHUNKS,
                blocks_per_chunk=BPC)


def _chunk_geometry(cfg):
    blk_rows = cfg["blk_rows"]
    geo, own0, base = [], 0, 0
    for b0, nb in CHUNK_BLOCKS:
        rows = sum(blk_rows[b0:b0 + nb])
        geo.append((own0, rows, base))
        own0 += rows
        base += cfg["NCORE"] * rows
    return geo


def _chunk_of_blk(nblk):
    m = np.empty(nblk, np.int64)
    for k, (b0, nb) in enumerate(CHUNK_BLOCKS):
        m[b0:b0 + nb] = k
    return m


def _prepare_host(x, edge_index, edge_weight, W1, b1, W2, b2, Wres, bres,
                  gamma1, beta1, gamma2, beta2, cfg):
    bf16 = ml_dtypes.bfloat16
    NPC, nblk = cfg["NPC"], cfg["nblk"]
    blk_rows = cfg["blk_rows"]
    x = np.asarray(x, np.float32)
    ei = np.asarray(edge_index).astype(np.int64)
    ew = np.asarray(edge_weight, np.float32)
    loop = np.arange(N, dtype=np.int64)
    src = np.concatenate([ei[0], loop])
    dst = np.concatenate([ei[1], loop])
    w = np.concatenate([ew, np.ones(N, np.float32)])
    deg = np.bincount(dst, weights=w, minlength=N).astype(np.float32)
    dinv = np.where(deg > 0, 1.0 / np.sqrt(deg), 0.0).astype(np.float32)
    norm = (dinv[src] * w * dinv[dst]).astype(np.float32)

    indeg = np.bincount(dst, minlength=N)
    order = np.argsort(-indeg, kind="stable")
    node_core = np.empty(N, np.int32)
    node_blk = np.empty(N, np.int32)
    node_slot = np.empty(N, np.int32)
    pos = 0
    for b in range(nblk):
        rb = blk_rows[b]
        chunk = order[pos: pos + NCORE * rb]
        pos += NCORE * rb
        i = np.arange(len(chunk))
        node_core[chunk] = i % NCORE
        node_blk[chunk] = b
        node_slot[chunk] = i // NCORE
    own_row = node_blk * P + node_slot
    blk_row_start = np.concatenate([[0], np.cumsum(blk_rows)]).astype(np.int64)
    own_crow = blk_row_start[node_blk] + node_slot

    geo = _chunk_geometry(cfg)
    cob = _chunk_of_blk(nblk)
    hpos = np.empty(N, np.int64)
    for k, (own0, rows, base) in enumerate(geo):
        m = cob[node_blk] == k
        hpos[m] = base + node_core[m] * rows + (own_crow[m] - own0)

    gkey = node_core[dst] * nblk + node_blk[dst]
    eorder = np.argsort(gkey, kind="stable")
    src_s, dst_s, norm_s = src[eorder], dst[eorder], norm[eorder]
    counts_all = np.bincount(gkey[eorder], minlength=NCORE * nblk)
    gstart = np.concatenate([[0], np.cumsum(counts_all)]).astype(np.int64)

    diag_tiles = np.zeros((NCORE, nblk, P, P), np.float32)
    grp_srcs, grp_edges = {}, {}
    for c in range(NCORE):
        for b in range(nblk):
            g = c * nblk + b
            s0, s1 = gstart[g], gstart[g + 1]
            sg, dg, ng = src_s[s0:s1], dst_s[s0:s1], norm_s[s0:s1]
            dcol = node_slot[dg]
            is_diag = (node_core[sg] == c) & (node_blk[sg] == b)
            np.add.at(diag_tiles[c, b],
                      (node_slot[sg[is_diag]], dcol[is_diag]), ng[is_diag])
            sgg, dgg, ngg = sg[~is_diag], dcol[~is_diag], ng[~is_diag]
            uniq, inv = np.unique(sgg, return_inverse=True)
            grp_srcs[(c, b)] = uniq
            grp_edges[(c, b)] = (inv, dgg, ngg)

    T = [max(1, int(np.ceil(max(len(grp_srcs[(c, b)]) for c in range(NCORE))
                            / P))) for b in range(nblk)]
    T_tot = int(sum(T))
    off = np.concatenate([[0], np.cumsum(T)]).astype(np.int64)
    cfg = dict(cfg, T=T, T_tot=T_tot, off=off.tolist())

    x_bf = x.astype(bf16)
    common = {
        "xg": x_bf,
        "w1": np.asarray(W1, np.float32).astype(bf16),
        "wres": np.asarray(Wres, np.float32).astype(bf16),
        "w2": np.ascontiguousarray(
            np.asarray(W2, np.float32).astype(bf16).reshape(2, DH // 2, DH)),
        "b1": np.asarray(b1, np.float32).astype(bf16).reshape(1, DH),
        "bres": np.asarray(bres, np.float32).astype(bf16).reshape(1, DH),
        "b2": np.asarray(b2, np.float32).astype(bf16).reshape(1, DH),
        "g1": np.asarray(gamma1, np.float32).reshape(1, DH),
        "be1": np.asarray(beta1, np.float32).reshape(1, DH),
        "g2": np.asarray(gamma2, np.float32).reshape(1, DH),
        "be2": np.asarray(beta2, np.float32).reshape(1, DH),
    }

    in_maps = []
    for c in range(NCORE):
        oh = np.zeros((P, T_tot, P), np.float32)
        idx1 = np.zeros((P, T_tot * 8), np.int16)
        idx2 = np.zeros((P, T_tot * 8), np.int16)
        xownT = np.zeros((DIN, nblk * P), bf16)
        xrows = np.zeros((P, nblk, DIN), bf16)
        nodes_c = np.where(node_core == c)[0]
        xownT[:, own_row[nodes_c]] = x[nodes_c].T.astype(bf16)
        xrows[node_slot[nodes_c], node_blk[nodes_c], :] = x_bf[nodes_c]
        for b in range(nblk):
            uniq = grp_srcs[(c, b)]
            inv, dcol, nrm = grp_edges[(c, b)]
            np.add.at(oh, (inv % P, off[b] + inv // P, dcol), nrm)
            n = len(uniq)
            i = np.arange(n)
            col = 8 * off[b] + i // 16
            row = i % 16
            u2 = hpos[uniq]
            for gg in range(8):
                idx1[gg * 16 + row, col] = uniq
                idx2[gg * 16 + row, col] = u2
        in_maps.append(dict(common,
                            oh=oh.astype(bf16),
                            diag=np.ascontiguousarray(
                                diag_tiles[c].transpose(1, 0, 2)).astype(bf16),
                            idx1=idx1, idx2=idx2,
                            xownT=xownT, xrows=xrows))

    def reassemble(core_outs):
        full = np.empty((N, DH), np.float32)
        for c in range(NCORE):
            mine = np.where(node_core == c)[0]
            full[mine] = np.asarray(core_outs[c])[own_crow[mine]]
        return full

    return in_maps, cfg, reassemble


def _build_kernel(cfg, reps=1):
    import concourse.bacc as bacc
    import concourse.tile as tile
    from concourse import mybir

    nblk = cfg["nblk"]
    T, T_tot, off = cfg["T"], cfg["T_tot"], cfg["off"]
    blk_rows = cfg["blk_rows"]
    blk_row_start = np.concatenate([[0], np.cumsum(blk_rows)]).astype(int)
    geo = _chunk_geometry(cfg)
    cob = _chunk_of_blk(nblk)
    chunk_last = {b0 + nb - 1: k for k, (b0, nb) in enumerate(CHUNK_BLOCKS)}
    nch = len(CHUNK_BLOCKS)
    NPC = cfg["NPC"]
    bf = mybir.dt.bfloat16
    f32 = mybir.dt.float32
    AF = mybir.ActivationFunctionType

    import os
    _scr = int(os.environ.get("BASS_SCRATCH", "16384"))
    nc = bacc.Bacc(dynamic_dma_scratch_size=_scr)
    xg = nc.dram_tensor("xg", [N, DIN], bf, kind="ExternalInput")
    xownT_in = nc.dram_tensor("xownT", [DIN, nblk * P], bf, kind="ExternalInput")
    xrows_in = nc.dram_tensor("xrows", [P, nblk, DIN], bf, kind="ExternalInput")
    oh_in = nc.dram_tensor("oh", [P, T_tot, P], bf, kind="ExternalInput")
    diag_in = nc.dram_tensor("diag", [P, nblk, P], bf, kind="ExternalInput")
    idx1_in = nc.dram_tensor("idx1", [P, T_tot * 8], mybir.dt.int16, kind="ExternalInput")
    idx2_in = nc.dram_tensor("idx2", [P, T_tot * 8], mybir.dt.int16, kind="ExternalInput")
    w1_in = nc.dram_tensor("w1", [DIN, DH], bf, kind="ExternalInput")
    wres_in = nc.dram_tensor("wres", [DIN, DH], bf, kind="ExternalInput")
    w2_in = nc.dram_tensor("w2", [2, DH // 2, DH], bf, kind="ExternalInput")
    b1_in = nc.dram_tensor("b1", [1, DH], bf, kind="ExternalInput")
    bres_in = nc.dram_tensor("bres", [1, DH], bf, kind="ExternalInput")
    b2_in = nc.dram_tensor("b2", [1, DH], bf, kind="ExternalInput")
    g1_in = nc.dram_tensor("g1", [1, DH], f32, kind="ExternalInput")
    be1_in = nc.dram_tensor("be1", [1, DH], f32, kind="ExternalInput")
    g2_in = nc.dram_tensor("g2", [1, DH], f32, kind="ExternalInput")
    be2_in = nc.dram_tensor("be2", [1, DH], f32, kind="ExternalInput")

    hown_ks = [nc.dram_tensor(f"hown_k{k}", [geo[k][1], DH], bf)
               for k in range(nch)]
    hfull_d = nc.dram_tensor("hfull_d", [N, DH], bf, addr_space="Shared")
    out_d = nc.dram_tensor("out", [NPC, DH], f32, kind="ExternalOutput")

    with tile.TileContext(nc) as tc:
        with tc.tile_pool(name="consts", bufs=1) as consts, \
             tc.tile_pool(name="gath1", bufs=3) as gath1p, \
             tc.tile_pool(name="gath2", bufs=3) as gath2p, \
             tc.tile_pool(name="acc", bufs=3) as accp, \
             tc.tile_pool(name="dve", bufs=3) as dvep, \
             tc.tile_pool(name="hpool", bufs=1) as hpool, \
             tc.tile_pool(name="pagg", bufs=2, space="PSUM") as pagg, \
             tc.tile_pool(name="pout", bufs=3, space="PSUM") as pout:

            # idx loads alone on the sync HWDGE ring (gathers start early);
            # all other constants go through the scalar HWDGE ring.
            idx1_t = consts.tile([P, T_tot * 8], mybir.dt.int16)
            nc.sync.dma_start(out=idx1_t[:], in_=idx1_in[:])
            idx2_t = consts.tile([P, T_tot * 8], mybir.dt.int16)
            nc.sync.dma_start(out=idx2_t[:], in_=idx2_in[:])
            diag_t = consts.tile([P, nblk, P], bf)
            nc.scalar.dma_start(out=diag_t[:], in_=diag_in[:])
            xrows_t = consts.tile([P, nblk, DIN], bf)
            nc.scalar.dma_start(out=xrows_t[:], in_=xrows_in[:])
            oh_t = []
            for b in range(nblk):
                ot = consts.tile([P, T[b], P], bf, tag=f"oh{b}")
                nc.scalar.dma_start(out=ot[:], in_=oh_in[:, off[b]:off[b] + T[b], :])
                oh_t.append(ot)
            xownT_t = consts.tile([DIN, nblk * P], bf)
            nc.scalar.dma_start(out=xownT_t[:], in_=xownT_in[:])
            w1_t = consts.tile([DIN, DH], bf)
            nc.scalar.dma_start(out=w1_t[:], in_=w1_in[:])
            wres_t = consts.tile([DIN, DH], bf)
            nc.scalar.dma_start(out=wres_t[:], in_=wres_in[:])
            w2_t = consts.tile([DH // 2, 2, DH], bf)
            nc.scalar.dma_start(out=w2_t[:], in_=w2_in.rearrange("k p d -> p k d"))
            brow = {}
            for name, t_in in (("b1", b1_in), ("bres", bres_in), ("b2", b2_in)):
                bt = consts.tile([1, DH], bf, tag=f"bias_{name}")
                nc.scalar.dma_start(out=bt[:], in_=t_in[:])
                brow[name] = bt
            gb = {}
            for name, t_in in (("g1", g1_in), ("be1", be1_in),
                               ("g2", g2_in), ("be2", be2_in)):
                gt = consts.tile([P, DH], f32, tag=f"gb_{name}")
                nc.scalar.dma_start(out=gt[:], in_=t_in[:].to_broadcast([P, DH]))
                gb[name] = gt
            ones_t = consts.tile([1, P], bf)
            nc.vector.memset(ones_t[:], 1.0)
            eps_t = consts.tile([P, 1], f32)
            nc.vector.memset(eps_t[:], LN_EPS)

            h_own = hpool.tile([P, nblk, DH], bf)

            def layer_norm(pre, gamma, beta, out_tile):
                mv = dvep.tile([P, 2], f32, tag="ln_mv")
                stats = dvep.tile([P, 6], f32, tag="ln_stats")
                nc.vector.bn_stats(out=stats[:], in_=pre[:])
                nc.vector.bn_aggr(out=mv[:], in_=stats[:])
                rstd = dvep.tile([P, 1], f32, tag="ln_rstd")
                nc.scalar.activation(out=rstd[:], in_=mv[:, 1:2], func=AF.Sqrt,
                                     bias=eps_t[:], scale=1.0)
                nc.vector.reciprocal(out=rstd[:], in_=rstd[:])
                nmr = dvep.tile([P, 1], f32, tag="ln_nmr")
                nc.vector.tensor_tensor(out=nmr[:], in0=mv[:, 0:1], in1=rstd[:],
                                        op=mybir.AluOpType.mult)
                nc.vector.tensor_scalar(out=nmr[:], in0=nmr[:], scalar1=-1.0,
                                        scalar2=None, op0=mybir.AluOpType.mult)
                tnorm = dvep.tile([P, DH], f32, tag="ln_tnorm")
                nc.scalar.activation(out=tnorm[:], in_=pre[:], func=AF.Identity,
                                     bias=nmr[:], scale=rstd[:])
                tg = dvep.tile([P, DH], f32, tag="ln_tg")
                nc.vector.tensor_tensor(out=tg[:], in0=tnorm[:], in1=gamma[:],
                                        op=mybir.AluOpType.mult)
                nc.vector.tensor_tensor(out=out_tile, in0=tg[:], in1=beta[:],
                                        op=mybir.AluOpType.add)

            def ag_chunk(k):
                own0, rows, base = geo[k]
                return nc.gpsimd.collective_compute(
                    "AllGather", mybir.AluOpType.bypass,
                    ins=[hown_ks[k][:]],
                    outs=[hfull_d[base: base + NCORE * rows, :]],
                    replica_groups=[list(range(NCORE))])

            def hop1(interleave_ag=False):
                pending_ag = []
                for b in range(nblk):
                    tb = T[b]
                    rb = blk_rows[b]
                    gt = gath1p.tile([P, tb, DIN], bf, tag="g1")
                    g_inst = nc.gpsimd.dma_gather(
                        out_ap=gt[:], in_ap=xg[:],
                        idxs_ap=idx1_t[:, 8 * off[b]: 8 * (off[b] + tb)],
                        num_idxs=tb * P, num_idxs_reg=tb * P, elem_size=DIN,
                        single_packet=False)
                    while pending_ag and pending_ag[0][0] <= b:
                        tile.add_dep_helper(g_inst.ins, pending_ag.pop(0)[1].ins,
                                            sync=False,
                                            reason="trigger AG before this gather")
                    ps_a = pagg.tile([P, P], f32, space="PSUM", tag="agg0")
                    nc.tensor.matmul(out=ps_a[:DIN, :],
                                     lhsT=xrows_t[:rb, b, :], rhs=diag_t[:rb, b, :],
                                     start=True, stop=False)
                    for t in range(tb):
                        nc.tensor.matmul(out=ps_a[:DIN, :],
                                         lhsT=gt[:, t, :], rhs=oh_t[b][:, t, :],
                                         start=False, stop=(t == tb - 1))
                    a1t = accp.tile([DIN, P], bf, tag="a1t")
                    nc.vector.tensor_copy(out=a1t[:], in_=ps_a[:DIN, :])
                    ps_h = pout.tile([P, DH], f32, space="PSUM", tag="po")
                    nc.tensor.matmul(out=ps_h[:], lhsT=ones_t[:], rhs=brow["b1"][:],
                                     start=True, stop=False)
                    nc.tensor.matmul(out=ps_h[:], lhsT=a1t[:], rhs=w1_t[:],
                                     start=False, stop=True)
                    ps_r = pout.tile([P, DH], f32, space="PSUM", tag="po")
                    nc.tensor.matmul(out=ps_r[:], lhsT=ones_t[:], rhs=brow["bres"][:],
                                     start=True, stop=False)
                    nc.tensor.matmul(out=ps_r[:],
                                     lhsT=xownT_t[:, b * P:(b + 1) * P], rhs=wres_t[:],
                                     start=False, stop=True)
                    delta = dvep.tile([P, DH], f32, tag="delta")
                    nc.scalar.activation(out=delta[:], in_=ps_h[:], func=AF.Relu)
                    pre = dvep.tile([P, DH], f32, tag="pre1")
                    nc.vector.tensor_tensor(out=pre[:], in0=delta[:], in1=ps_r[:],
                                            op=mybir.AluOpType.add)
                    layer_norm(pre, gb["g1"], gb["be1"], h_own[:, b, :])
                    kb = int(cob[b])
                    r0 = blk_row_start[b] - geo[kb][0]
                    nc.sync.dma_start(out=hown_ks[kb][r0: r0 + rb, :],
                                      in_=h_own[:rb, b, :])
                    if interleave_ag and b in chunk_last and b < nblk - 1:
                        pending_ag.append((min(b + 4, nblk - 1),
                                           ag_chunk(chunk_last[b])))

            def hop2():
                for b in range(nblk):
                    tb = T[b]
                    rb = blk_rows[b]
                    gt = gath2p.tile([P, tb, DH], bf, tag="g2")
                    nc.gpsimd.dma_gather(
                        out_ap=gt[:], in_ap=hfull_d[:],
                        idxs_ap=idx2_t[:, 8 * off[b]: 8 * (off[b] + tb)],
                        num_idxs=tb * P, num_idxs_reg=tb * P, elem_size=DH,
                        single_packet=False)
                    ps_a = pagg.tile([P, P], f32, space="PSUM", tag="agg0")
                    ps_b = pagg.tile([P, P], f32, space="PSUM", tag="agg1")
                    nc.tensor.matmul(out=ps_a[:], lhsT=h_own[:rb, b, 0:DH // 2],
                                     rhs=diag_t[:rb, b, :], start=True, stop=False)
                    for t in range(tb):
                        nc.tensor.matmul(out=ps_a[:], lhsT=gt[:, t, 0:DH // 2],
                                         rhs=oh_t[b][:, t, :],
                                         start=False, stop=(t == tb - 1))
                    nc.tensor.matmul(out=ps_b[:], lhsT=h_own[:rb, b, DH // 2:DH],
                                     rhs=diag_t[:rb, b, :], start=True, stop=False)
                    for t in range(tb):
                        nc.tensor.matmul(out=ps_b[:], lhsT=gt[:, t, DH // 2:DH],
                                         rhs=oh_t[b][:, t, :],
                                         start=False, stop=(t == tb - 1))
                    a2a = accp.tile([DH // 2, P], bf, tag="a2a")
                    a2b = accp.tile([DH // 2, P], bf, tag="a2b")
                    nc.vector.tensor_copy(out=a2a[:], in_=ps_a[:])
                    nc.vector.tensor_copy(out=a2b[:], in_=ps_b[:])
                    ps_o = pout.tile([P, DH], f32, space="PSUM", tag="po")
                    nc.tensor.matmul(out=ps_o[:], lhsT=ones_t[:], rhs=brow["b2"][:],
                                     start=True, stop=False)
                    nc.tensor.matmul(out=ps_o[:], lhsT=a2a[:], rhs=w2_t[:, 0, :],
                                     start=False, stop=False)
                    nc.tensor.matmul(out=ps_o[:], lhsT=a2b[:], rhs=w2_t[:, 1, :],
                                     start=False, stop=True)
                    pre = dvep.tile([P, DH], f32, tag="pre2")
                    nc.vector.tensor_tensor(out=pre[:], in0=ps_o[:],
                                            in1=h_own[:, b, :],
                                            op=mybir.AluOpType.add)
                    outb = dvep.tile([P, DH], f32, tag="out2")
                    layer_norm(pre, gb["g2"], gb["be2"], outb[:])
                    r0 = blk_row_start[b]
                    nc.sync.dma_start(out=out_d[r0: r0 + rb, :], in_=outb[:rb, :])

            hop1(interleave_ag=True)
            ag_chunk(nch - 1)
            hop2()
            if reps > 1:
                # collectives can't sit inside control flow; rep iterations
                # recompute both hops against the already-exchanged h_full
                # (results identical, used for repeat-timing only).
                with tc.For_i(0, reps - 1, 1):
                    hop1()
                    hop2()

    nc.finalize()
    return nc


def _kernel_numpy(x, edge_index, edge_weight, W1, b1, W2, b2, Wres, bres,
                  gamma1, beta1, gamma2, beta2):
    x = np.asarray(x, np.float32)
    n = x.shape[0]
    ei = np.asarray(edge_index).astype(np.int64)
    loop = np.arange(n)
    src = np.concatenate([ei[0], loop])
    dst = np.concatenate([ei[1], loop])
    w = np.concatenate([np.asarray(edge_weight, np.float32),
                        np.ones(n, np.float32)])

    def seg(vals, idx):
        out = np.zeros((n,) + vals.shape[1:], np.float32)
        np.add.at(out, idx, vals)
        return out

    deg = seg(w, dst)
    dinv = np.where(deg > 0, 1 / np.sqrt(deg), 0).astype(np.float32)
    nrm = dinv[src] * w * dinv[dst]

    def conv(h, W, b):
        return seg(nrm[:, None] * (h @ W)[src], dst) + b

    def ln(v, g, be):
        mu = v.mean(-1, keepdims=True)
        var = ((v - mu) ** 2).mean(-1, keepdims=True)
        return (v - mu) / np.sqrt(var + LN_EPS) * g + be

    delta = np.maximum(conv(x, np.asarray(W1, np.float32), b1), 0)
    h = ln(x @ np.asarray(Wres, np.float32) + bres + delta, gamma1, beta1)
    delta2 = conv(h, np.asarray(W2, np.float32), b2)
    return ln(h + delta2, gamma2, beta2).astype(np.float32)


def _install_ntff_hook():
    """Register the NTFF profile hook bass_utils expects under axon (the
    image's antenv package lacks axon_hooks; provide it in-process)."""
    import types
    if "antenv.axon_hooks" in sys.modules:
        return
    mod = types.ModuleType("antenv.axon_hooks")
    state = {"h": None}
    mod.set_axon_ntff_profile_hook = lambda h: state.__setitem__("h", h)
    mod.get_axon_ntff_profile_hook = lambda: state["h"]
    sys.modules["antenv.axon_hooks"] = mod
    try:
        from trn_agent_boot.trn_boot import _ntff_profile_via_ctypes
        h = _ntff_profile_via_ctypes("/opt/axon/libaxon_pjrt.so")
        if h is not None:
            mod.set_axon_ntff_profile_hook(h)
        import concourse.bass_utils as bu
        bu.upload_artifacts = lambda tmpdir: f"local:{tmpdir}"
    except Exception:
        pass


def _run_device(inputs, trace=False):
    from concourse.bass_utils import run_bass_kernel_spmd
    if trace:
        _install_ntff_hook()
    cfg = _make_cfg()
    in_maps, cfg, reassemble = _prepare_host(
        inputs["x"], inputs["edge_index"], inputs["edge_weight"],
        inputs["W1"], inputs["b1"], inputs["W2"], inputs["b2"],
        inputs["Wres"], inputs["bres"],
        inputs["gamma1"], inputs["beta1"], inputs["gamma2"], inputs["beta2"],
        cfg)
    nc = _build_kernel(cfg)
    res = run_bass_kernel_spmd(nc, in_maps, list(range(NCORE)), trace=trace)
    out = reassemble([res.results[c]["out"] for c in range(NCORE)])
    return out, res.exec_time_ns


def run_traced(inputs):
    """Returns (full_output, hw_exec_time_ns measured via neuron-profile)."""
    return _run_device(inputs, trace=True)


def kernel(x, edge_index, edge_weight, W1, b1, W2, b2, Wres, bres,
           gamma1, beta1, gamma2, beta2):
    inputs = dict(x=x, edge_index=edge_index, edge_weight=edge_weight,
                  W1=W1, b1=b1, W2=W2, b2=b2, Wres=Wres, bres=bres,
                  gamma1=gamma1, beta1=beta1, gamma2=gamma2, beta2=beta2)
    try:
        out, _ = _run_device(inputs, trace=False)
        return out
    except Exception as e:  # device path unavailable -> numpy fallback
        print(f"kernel: device path failed ({type(e).__name__}: {e}); "
              f"falling back to numpy", file=sys.stderr)
        return _kernel_numpy(**inputs)
